# revision 8
# baseline (speedup 1.0000x reference)
"""GQA attention kernel for Trainium2, 8 NeuronCores.

Sharding: batch (2) x head-groups (4). Core c handles batch c//4... see bottom.
Each core: 8 q heads / 2 kv heads, full 2048-seq causal attention + partial
output projection (Wo split on input dim); host sums the 4 partials per batch.

Layout strategy per core:
  - xT via PE transposes (d on partitions) feeds all projections.
  - Q,K computed transposed ([head*64, s]); V natural ([s, 64*2+ones]).
  - Scores computed transposed: S.T[sk,sq] = (KT tile).T @ QT chunk; exp via
    ACT (scale=1/8 fused); causal diag masks multiplied in (host-provided).
  - PV: O.T[65, sq] accumulated with V augmented by a ones column -> row 64 is
    the softmax denominator. Normalize via reciprocal + outer-product
    broadcast matmul + DVE multiply.
  - Output projection consumes O.T directly as lhsT.
Matmuls run in float32r (full PE rate) by default.
"""

import sys
import numpy as np

sys.path.insert(0, "/opt/trn_rl_repo")

import concourse.bass as bass  # noqa: E402
import concourse.mybir as mybir  # noqa: E402
import concourse.tile as tile  # noqa: E402
from concourse import bacc  # noqa: E402
from concourse.masks import make_identity  # noqa: E402

B, S, D = 2, 2048, 2048
NQ, NKV, HD = 32, 8, 64
THETA = 10000.0
P = 128
SC = 512              # s-chunk (matmul free dim)
NSC = S // SC         # 4
DT = D // P           # 16 d-tiles
NCORES = 8
GROUPS = 4            # head-groups (cores per batch)
QH_L = NQ // GROUPS   # 8 q heads per core
KV_L = NKV // GROUPS  # 2 kv heads per core
QO = QH_L * HD        # 512 q-proj out dim per core
KO = KV_L * HD        # 128 kv-proj out dim per core

F32 = mybir.dt.float32
USE_F32R = True
MMDT = mybir.dt.float32r if USE_F32R else F32

AF = mybir.ActivationFunctionType


def _mm(t):
    """View an SBUF AP in the matmul dtype."""
    return t.bitcast(MMDT) if t.dtype != MMDT else t


def build_program():
    nc = bacc.Bacc(None)
    xb = nc.declare_dram_parameter("xb", [S, D], F32, isOutput=False)
    wqT = nc.declare_dram_parameter("wqT", [D, QO], F32, isOutput=False)
    wkT = nc.declare_dram_parameter("wkT", [D, KO], F32, isOutput=False)
    wvT = nc.declare_dram_parameter("wvT", [D, KO], F32, isOutput=False)
    woT = nc.declare_dram_parameter("woT", [QO, D], F32, isOutput=False)
    cs = nc.declare_dram_parameter("cs", [P, S], F32, isOutput=False)
    sn = nc.declare_dram_parameter("sn", [P, S], F32, isOutput=False)
    dmask = nc.declare_dram_parameter("dmask", [P, NSC * SC], F32, isOutput=False)
    y = nc.declare_dram_parameter("y", [S, D], F32, isOutput=True)

    with tile.TileContext(nc) as tc:
        _build_tile(nc, tc, xb, wqT, wkT, wvT, woT, cs, sn, dmask, y)
    return nc


def _build_tile(nc, tc, xb, wqT, wkT, wvT, woT, cs, sn, dmask, y):
    from contextlib import ExitStack

    ctx = ExitStack()
    with ctx:
        if USE_F32R:
            ctx.enter_context(nc.allow_low_precision(
                reason="float32r matmul operands (11-bit mantissa) by design"))
        persist = ctx.enter_context(tc.tile_pool(name="persist", bufs=1))

        # persistent tiles
        qtr = [persist.tile([P, S], MMDT, tag=f"qtr{i}", name=f"qtr{i}") for i in range(QO // P)]
        ktr = persist.tile([P, S], MMDT, tag="ktr")
        # V augmented: [s-tile, 65*KV_L]; col 64/129 = ones (denominator trick)
        vaug = [persist.tile([P, 65 * KV_L], MMDT, tag=f"vaug{t}", name=f"vaug{t}")
                for t in range(S // P)]
        ot = [persist.tile([P, S], MMDT, tag=f"ot{i}", name=f"ot{i}") for i in range(QO // P)]
        ident = persist.tile([P, P], F32, tag="ident")
        ones64 = persist.tile([1, HD], MMDT, tag="ones64")
        dmask_sb = persist.tile([P, NSC * SC], F32, tag="dmask")

        ones_f32 = persist.tile([P, HD], F32, tag="ones_f32")
        make_identity(nc, ident[:])
        nc.gpsimd.memset(ones_f32[:], 1.0)
        nc.scalar.activation(ones64[:], ones_f32[0:1, :], AF.Copy)
        for t in range(S // P):
            for g in range(KV_L):
                nc.scalar.activation(
                    vaug[t][:, g * 65 + HD: g * 65 + HD + 1],
                    ones_f32[:, 0:1], AF.Copy)
        nc.sync.dma_start(dmask_sb[:], dmask[:])

        # ---------------- phase 2: xT + QKV projections ----------------
        with tc.tile_pool(name="p2w", bufs=1) as p2w, \
             tc.tile_pool(name="wstage", bufs=4) as wstage, \
             tc.tile_pool(name="xstage", bufs=2) as xstage, \
             tc.tile_pool(name="xtc", bufs=1) as xtcp, \
             tc.tile_pool(name="ps_tr", bufs=2, space="PSUM") as ps_tr, \
             tc.tile_pool(name="ps_qkv", bufs=2, space="PSUM") as ps_qkv:

            wq_sb = [p2w.tile([P, QO], MMDT, tag=f"wq{d}", name=f"wq{d}") for d in range(DT)]
            wk_sb = [p2w.tile([P, KO], MMDT, tag=f"wk{d}", name=f"wk{d}") for d in range(DT)]
            wv_sb = [p2w.tile([P, KO], MMDT, tag=f"wv{d}", name=f"wv{d}") for d in range(DT)]
            for d in range(DT):
                st = wstage.tile([P, QO], F32, tag="wst")
                nc.sync.dma_start(st[:], wqT[d * P:(d + 1) * P, :])
                nc.scalar.activation(wq_sb[d][:], st[:], AF.Copy)
                st2 = wstage.tile([P, QO], F32, tag="wst")
                nc.sync.dma_start(st2[:, :KO], wkT[d * P:(d + 1) * P, :])
                nc.scalar.activation(wk_sb[d][:], st2[:, :KO], AF.Copy)
                st3 = wstage.tile([P, QO], F32, tag="wst")
                nc.sync.dma_start(st3[:, :KO], wvT[d * P:(d + 1) * P, :])
                nc.scalar.activation(wv_sb[d][:], st3[:, :KO], AF.Copy)

            xtc = [xtcp.tile([P, SC], MMDT, tag=f"xtc{d}", name=f"xtc{d}") for d in range(DT)]
            for c in range(NSC):
                # transpose x rows chunk -> xT columns chunk
                for r in range(SC // P):
                    xs = xstage.tile([P, D], F32, tag="xs")
                    row0 = c * SC + r * P
                    nc.sync.dma_start(xs[:], xb[row0:row0 + P, :])
                    for d in range(DT):
                        pt = ps_tr.tile([P, P], F32, tag="ps_tr")
                        nc.tensor.transpose(pt[:], xs[:, d * P:(d + 1) * P], ident[:])
                        nc.scalar.activation(
                            xtc[d][:, r * P:(r + 1) * P], pt[:], AF.Copy)
                # Q projection: QT[o, s-chunk]
                for o in range(QO // P):
                    ps = ps_qkv.tile([P, SC], F32, tag="ps_qkv")
                    for d in range(DT):
                        nc.tensor.matmul(
                            ps[:], wq_sb[d][:, o * P:(o + 1) * P], xtc[d][:],
                            start=(d == 0), stop=(d == DT - 1))
                    nc.scalar.activation(
                        qtr[o][:, c * SC:(c + 1) * SC], ps[:], AF.Copy)
                # K projection
                ps = ps_qkv.tile([P, SC], F32, tag="ps_qkv")
                for d in range(DT):
                    nc.tensor.matmul(ps[:], wk_sb[d][:], xtc[d][:],
                                     start=(d == 0), stop=(d == DT - 1))
                nc.scalar.activation(
                    ktr[:, c * SC:(c + 1) * SC], ps[:], AF.Copy)
                # V projection (natural layout, into augmented tiles)
                for r in range(SC // P):
                    ps = ps_qkv.tile([P, SC], F32, tag="ps_qkv")
                    for d in range(DT):
                        nc.tensor.matmul(
                            ps[:, :KO], xtc[d][:, r * P:(r + 1) * P], wv_sb[d][:],
                            start=(d == 0), stop=(d == DT - 1))
                    vt = vaug[c * (SC // P) + r]
                    for g in range(KV_L):
                        nc.scalar.activation(
                            vt[:, g * 65:g * 65 + HD], ps[:, g * HD:(g + 1) * HD],
                            AF.Copy)

        # ---------------- phase 3: RoPE on QT, KT ----------------
        with tc.tile_pool(name="p3", bufs=1) as p3, \
             tc.tile_pool(name="rsc", bufs=2) as rsc:
            cs_sb = p3.tile([P, S], F32, tag="cs")
            sn_sb = p3.tile([P, S], F32, tag="sn")
            nc.sync.dma_start(cs_sb[:], cs[:])
            nc.sync.dma_start(sn_sb[:], sn[:])
            H2 = HD // 2
            for t in qtr + [ktr]:
                tf = t[:].bitcast(F32)
                rt = rsc.tile([P, S], F32, tag="rt")
                # rotate-half blocks: rows [0:32)<- -rows[32:64) etc.
                for base in (0, HD):
                    nc.scalar.activation(rt[base:base + H2, :],
                                         tf[base + H2:base + HD, :],
                                         AF.Copy, scale=-1.0)
                    nc.scalar.activation(rt[base + H2:base + HD, :],
                                         tf[base:base + H2, :], AF.Copy)
                nc.vector.tensor_mul(rt[:], rt[:], sn_sb[:])
                nc.vector.tensor_mul(t[:], tf, cs_sb[:])
                nc.vector.tensor_add(t[:], t[:].bitcast(F32), rt[:])

        # ---------------- phase 4: attention ----------------
        with tc.tile_pool(name="ptp", bufs=18) as ptp, \
             tc.tile_pool(name="pttmp", bufs=3) as pttmp, \
             tc.tile_pool(name="rcp", bufs=4) as rcpp, \
             tc.tile_pool(name="ps_st", bufs=4, space="PSUM") as ps_st, \
             tc.tile_pool(name="ps_o", bufs=2, space="PSUM") as ps_op, \
             tc.tile_pool(name="ps_b", bufs=2, space="PSUM") as ps_bp:
            for h in range(QH_L):
                kv = h // (QH_L // KV_L)
                qslice = qtr[h % 4][kv * HD:(kv + 1) * HD, :]
                kslice = ktr[kv * HD:(kv + 1) * HD, :]
                for c in range(NSC):
                    nst = (c + 1) * (SC // P)
                    pts = []
                    for kt in range(nst):
                        pss = ps_st.tile([P, SC], F32, tag="ps_st")
                        nc.tensor.matmul(
                            pss[:], kslice[:, kt * P:(kt + 1) * P],
                            qslice[:, c * SC:(c + 1) * SC],
                            start=True, stop=True)
                        pt = ptp.tile([P, SC], MMDT, tag="pt")
                        if kt >= (c + 1) * (SC // P) - (SC // P):
                            # diagonal tile: exp then mask-multiply
                            tmp = pttmp.tile([P, SC], F32, tag="pttmp")
                            nc.scalar.activation(tmp[:], pss[:], AF.Exp,
                                                 scale=0.125)
                            t = kt - c * (SC // P)
                            nc.vector.tensor_mul(
                                pt[:], tmp[:], dmask_sb[:, t * SC:(t + 1) * SC])
                        else:
                            nc.scalar.activation(pt[:], pss[:], AF.Exp,
                                                 scale=0.125)
                        pts.append(pt)
                    pso = ps_op.tile([P, SC], F32, tag="ps_o")
                    for kt in range(nst):
                        nc.tensor.matmul(
                            pso[:65, :], vaug[kt][:, kv * 65:(kv + 1) * 65],
                            pts[kt][:], start=(kt == 0), stop=(kt == nst - 1))
                    rcp = rcpp.tile([1, SC], MMDT, tag="rcp")
                    nc.vector.reciprocal(rcp[:], pso[HD:HD + 1, :])
                    psb = ps_bp.tile([HD, SC], F32, tag="ps_b")
                    nc.tensor.matmul(psb[:], ones64[:], rcp[:],
                                     start=True, stop=True)
                    osb = pttmp.tile([HD, SC], F32, tag="osb")
                    nc.scalar.activation(osb[:], pso[:HD, :], AF.Copy)
                    nc.vector.tensor_mul(
                        ot[h % 4][kv * HD:(kv + 1) * HD,
                                  c * SC:(c + 1) * SC],
                        osb[:], psb[:])

        # ---------------- phase 5: output projection ----------------
        with tc.tile_pool(name="p5w", bufs=1) as p5w, \
             tc.tile_pool(name="w5stage", bufs=3) as w5stage, \
             tc.tile_pool(name="yst", bufs=3) as ystp, \
             tc.tile_pool(name="ps_y", bufs=4, space="PSUM") as ps_y:
            wo_sb = [p5w.tile([P, D], MMDT, tag=f"wo{d}", name=f"wo{d}") for d in range(QO // P)]
            for d in range(QO // P):
                st = w5stage.tile([P, D], F32, tag="w5st")
                nc.sync.dma_start(st[:], woT[d * P:(d + 1) * P, :])
                nc.scalar.activation(wo_sb[d][:], st[:], AF.Copy)
            for s_t in range(S // P):
                for oc in range(D // SC):
                    ps = ps_y.tile([P, SC], F32, tag="ps_y")
                    for d in range(QO // P):
                        nc.tensor.matmul(
                            ps[:], ot[d][:, s_t * P:(s_t + 1) * P],
                            wo_sb[d][:, oc * SC:(oc + 1) * SC],
                            start=(d == 0), stop=(d == QO // P - 1))
                    ys = ystp.tile([P, SC], F32, tag="yst")
                    nc.scalar.activation(ys[:], ps[:], AF.Copy)
                    nc.sync.dma_start(
                        y[s_t * P:(s_t + 1) * P, oc * SC:(oc + 1) * SC], ys[:])


def _rope_tables():
    k = np.arange(0, HD, 2)[: HD // 2].astype(np.float64)
    inv_freq = 1.0 / (THETA ** (k / HD))
    pos = np.arange(S, dtype=np.float64)
    ang = pos[:, None] * inv_freq[None, :]          # [S, HD/2]
    ang = np.concatenate([ang, ang], axis=-1)       # [S, HD]
    cosT = np.cos(ang).T.astype(np.float32)         # [HD, S]
    sinT = np.sin(ang).T.astype(np.float32)
    return (np.ascontiguousarray(np.vstack([cosT, cosT])),
            np.ascontiguousarray(np.vstack([sinT, sinT])))


def _diag_masks():
    m = np.zeros((P, NSC * SC), dtype=np.float32)
    for t in range(NSC):
        for p in range(P):
            q0 = t * P + p
            if q0 < SC:
                m[p, t * SC + q0:(t + 1) * SC] = 1.0
    return m


HEAD_PERM = [0, 4, 1, 5, 2, 6, 3, 7]  # local head order in SBUF tiles


def _permute_heads_rows(w):
    # w: [QH_L*HD, ...] -> reorder 64-row head blocks by HEAD_PERM
    hs = w.reshape(QH_L, HD, -1)
    return hs[HEAD_PERM].reshape(w.shape)


def make_in_maps(x, Wq, Wk, Wv, Wo):
    csm, snm = _rope_tables()
    dm = _diag_masks()
    in_maps = []
    for core in range(NCORES):
        b, j = divmod(core, GROUPS)
        wq_s = _permute_heads_rows(Wq[j * QO:(j + 1) * QO, :])
        wo_s = _permute_heads_rows(
            np.ascontiguousarray(Wo[:, j * QO:(j + 1) * QO].T))
        in_maps.append({
            "xb": np.ascontiguousarray(x[b]),
            "wqT": np.ascontiguousarray(wq_s.T),
            "wkT": np.ascontiguousarray(Wk[j * KO:(j + 1) * KO, :].T),
            "wvT": np.ascontiguousarray(Wv[j * KO:(j + 1) * KO, :].T),
            "woT": np.ascontiguousarray(wo_s),
            "cs": csm, "sn": snm, "dmask": dm,
        })
    return in_maps


_prog_cache = {}


def _get_program():
    if "nc" not in _prog_cache:
        nc = build_program()
        nc.finalize()
        _prog_cache["nc"] = nc
    return _prog_cache["nc"]


def kernel(x, attention_mask, Wq, Wk, Wv, Wo, _trace=False):
    from concourse.bass_utils import run_bass_kernel_spmd

    x = np.asarray(x, dtype=np.float32)
    Wq = np.asarray(Wq, dtype=np.float32)
    Wk = np.asarray(Wk, dtype=np.float32)
    Wv = np.asarray(Wv, dtype=np.float32)
    Wo = np.asarray(Wo, dtype=np.float32)

    nc = _get_program()
    in_maps = make_in_maps(x, Wq, Wk, Wv, Wo)
    res = run_bass_kernel_spmd(nc, in_maps, list(range(NCORES)), trace=_trace)
    out = np.zeros((B, S, D), dtype=np.float32)
    for core in range(NCORES):
        b = core // GROUPS
        out[b] += res.results[core]["y"]
    if _trace:
        _prog_cache["last_result"] = res
    return out
